# revision 18
# baseline (speedup 1.0000x reference)
"""Trainium2 Bass kernel for nn_Attention_module_actor (dense_transformer).

8-way data parallel (4096 samples/core, 32 tiles of 128 samples on SBUF
partitions). The QKV projection + LayerNorm(H,N,D) + additive-attention
linears are algebraically fused host-side into one per-node linear map of
the augmented input [x_node, 1]; LayerNorms reduce to per-sample scalars
(r = rsqrt(var), mtil = mean*r) obtained from Cholesky-factor projections.
PE does the fused front-end, score transposes + alin, and the dense tail
(biases folded via ones-rows); ACT applies per-partition-scalar LN scaling
and transcendentals; DVE/GPSIMD do softmax, the per-sample bilinear P@V
(broadcast products + strided reduce), LN2+maxpool (max commutes with the
positive affine), and the output mask select.
"""

import numpy as np
from contextlib import ExitStack

import concourse.bass as bass
import concourse.bacc as bacc
import concourse.mybir as mybir
from concourse.tile import TileContext
from concourse.bass_utils import run_bass_kernel_spmd

B, N, F_IN, H, D = 32768, 14, 10, 3, 32
HID, OUT, A1, A2 = 100, 100, 9, 8
EPS = 1e-5
NCORES = 8
BC = B // NCORES
P = 128
HND = H * N * D
E1 = F_IN + 1
NF = 216

F16 = mybir.dt.float16
F32 = mybir.dt.float32


# ---------------------------------------------------------------------------
# host-side weight fusion
# ---------------------------------------------------------------------------

def _fuse_qk(p_w, p_b, g, bb, lw, lb):
    pw = p_w.reshape(F_IN, H, D)
    pb = p_b.reshape(H, D)
    Wt = np.einsum('ehd,htd,dc->ehtc', pw, g, lw)
    Wb = np.einsum('hd,htd,dc->htc', pb, g, lw)
    W = np.concatenate([Wt, Wb[None]], axis=0)
    NG = -np.einsum('htd,dc->htc', g, lw)
    CONST = np.einsum('htd,dc->htc', bb, lw) + lb[None, None, :]
    return W, NG, CONST


def _fuse_v(v_w, v_b, g, bb, l1w, l1b):
    vw = v_w.reshape(F_IN, H, D)
    vb = v_b.reshape(H, D)
    l1 = l1w.reshape(H, D, D)
    Wt = np.einsum('ehd,hcd,hdj->ehcj', vw, g, l1)
    Wb = np.einsum('hd,hcd,hdj->hcj', vb, g, l1)
    W = np.concatenate([Wt, Wb[None]], axis=0)
    NG = -np.einsum('hcd,hdj->hcj', g, l1)
    # softmax rows sum to 1 -> lin1_b/H per (h,c) sums back to lin1_b
    CONST = np.einsum('hcd,hdj->hcj', bb, l1) + l1b[None, None, :] / H
    return W, NG, CONST


def _stats_cols(p_w, p_b):
    Wbar = np.concatenate([p_w, p_b[None, :]], axis=0)
    Gram = Wbar @ Wbar.T
    reg = 1e-9 * (np.trace(Gram) / E1 + 1.0)
    L = np.linalg.cholesky(Gram + reg * np.eye(E1))
    M = Wbar.sum(axis=1)
    return L, M


def _host_prep(inputs):
    f = {k: np.asarray(v, np.float64) for k, v in inputs.items()
         if k != 'mask_vec'}
    mask = np.asarray(inputs['mask_vec'])

    Wq, NGq, Cq = _fuse_qk(f['q_w'], f['q_b'], f['qn_g'], f['qn_b'],
                           f['qlin_w'], f['qlin_b'])
    Wk, NGk, Ck = _fuse_qk(f['k_w'], f['k_b'], f['kn_g'], f['kn_b'],
                           f['klin_w'], f['klin_b'])
    Wv, NGv, Cv = _fuse_v(f['v_w'], f['v_b'], f['vn_g'], f['vn_b'],
                          f['lin1_w'], f['lin1_b'])
    Lq, Mq = _stats_cols(f['q_w'], f['q_b'])
    Lk, Mk = _stats_cols(f['k_w'], f['k_b'])
    Lv, Mv = _stats_cols(f['v_w'], f['v_b'])

    fw = np.zeros((44, 14 * NF), np.float64)
    for t in range(N):
        i = t % 4
        blk = np.zeros((E1, NF))
        blk[:, 0:42] = Wq[:, :, t, :].reshape(E1, 42)
        blk[:, 42:84] = Wk[:, :, t, :].reshape(E1, 42)
        blk[:, 84:180] = Wv[:, :, t, :].reshape(E1, 96)
        blk[:, 180:191] = Lq
        blk[:, 191:202] = Lk
        blk[:, 202:213] = Lv
        blk[:, 213] = Mq
        blk[:, 214] = Mk
        blk[:, 215] = Mv
        fw[11 * i:11 * i + 11, t * NF:(t + 1) * NF] = blk

    alin_w = f['alin_w']
    alin_bp = f['alin_b'] - alin_w.sum(axis=0)   # fold ELU's -1 shift
    albd = np.zeros((128, 112), np.float64)
    for g in range(8):
        albd[16 * g:16 * g + 14, 14 * g:14 * g + 14] = alin_w
        albd[16 * g + 14, 14 * g:14 * g + 14] = alin_bp

    rep = lambda a: np.broadcast_to(
        a.reshape(1, -1), (P, a.size)).astype(np.float16).copy()
    aug = lambda w, bvec: np.concatenate(
        [w, bvec[None, :]], axis=0).astype(np.float16)

    consts = {
        'fw': fw.astype(np.float16),
        'albd': albd.astype(np.float16),
        'ngq': rep(NGq.transpose(1, 0, 2)),
        'ngk': rep(NGk.transpose(1, 0, 2)),
        'cqk': rep((Cq + Ck).transpose(1, 0, 2)),
        'ngv': rep(NGv.transpose(2, 0, 1)),
        'cv': rep(Cv.transpose(2, 0, 1)),
        'ident': np.eye(128).astype(np.float16),
        'lin2a': aug(f['lin2_w'], f['lin2_b']),
        'l47a': aug(np.concatenate([f['l4_w'], f['l7_w']], 1),
                    np.concatenate([f['l4_b'] - f['l4_w'].sum(0),
                                    f['l7_b'] - f['l7_w'].sum(0)])),
        'l5a': aug(f['l5_w'], f['l5_b']),
        'l8a': aug(f['l8_w'], f['l8_b']),
        'l6a': aug(f['l6_w'], f['l6_b']),
        'l9a': aug(f['l9_w'], f['l9_b']),
        'negrep': np.full((P, A1), -1e8, np.float32),
    }

    x = np.asarray(inputs['x'], np.float32)
    in_maps = []
    for c in range(NCORES):
        xs = x[c * BC:(c + 1) * BC]
        xg = np.zeros((44, 4, BC), np.float16)
        for t in range(N):
            g, i = t // 4, t % 4
            xg[11 * i:11 * i + 10, g, :] = xs[:, t, :].T.astype(np.float16)
            xg[11 * i + 10, g, :] = 1.0
        im = {'xg': xg, 'mask': mask[c * BC:(c + 1) * BC].astype(np.uint8)}
        im.update(consts)
        in_maps.append(im)
    return in_maps


# ---------------------------------------------------------------------------
# device program
# ---------------------------------------------------------------------------

def _fe_split_ranges():
    """FE matmul col ranges split so no PSUM output crosses a 2KB bank."""
    out = []
    for t in range(N):
        b0, b1 = t * NF * 4, (t + 1) * NF * 4
        rngs = [(b0, b1)]
        if b0 // 2048 != (b1 - 1) // 2048:
            edge = ((b0 // 2048) + 1) * 2048
            rngs = [(b0, edge), (edge, b1)]
        for lo, hi in rngs:
            out.append((t, (lo - b0) // 4, (hi - b0) // 4))
    return out


def build_program(bc=BC):
    nt = bc // P
    nc = bacc.Bacc()

    xg_d = nc.dram_tensor('xg', [44, 4, bc], F16, kind='ExternalInput')
    mask_d = nc.dram_tensor('mask', [bc, A1], mybir.dt.uint8, kind='ExternalInput')
    fw_d = nc.dram_tensor('fw', [44, 14 * NF], F16, kind='ExternalInput')
    albd_d = nc.dram_tensor('albd', [128, 112], F16, kind='ExternalInput')
    ngq_d = nc.dram_tensor('ngq', [P, 588], F16, kind='ExternalInput')
    ngk_d = nc.dram_tensor('ngk', [P, 588], F16, kind='ExternalInput')
    cqk_d = nc.dram_tensor('cqk', [P, 588], F16, kind='ExternalInput')
    ngv_d = nc.dram_tensor('ngv', [P, 1344], F16, kind='ExternalInput')
    cv_d = nc.dram_tensor('cv', [P, 1344], F16, kind='ExternalInput')
    id_d = nc.dram_tensor('ident', [128, 128], F16, kind='ExternalInput')
    l2_d = nc.dram_tensor('lin2a', [33, OUT], F16, kind='ExternalInput')
    l47_d = nc.dram_tensor('l47a', [101, 200], F16, kind='ExternalInput')
    l5_d = nc.dram_tensor('l5a', [101, HID], F16, kind='ExternalInput')
    l8_d = nc.dram_tensor('l8a', [101, HID], F16, kind='ExternalInput')
    l6_d = nc.dram_tensor('l6a', [101, A1], F16, kind='ExternalInput')
    l9_d = nc.dram_tensor('l9a', [101, A2], F16, kind='ExternalInput')
    neg_d = nc.dram_tensor('negrep', [P, A1], F32, kind='ExternalInput')
    out1_d = nc.dram_tensor('out1', [bc, A1], F32, kind='ExternalOutput')
    out2_d = nc.dram_tensor('out2', [bc, A2], F32, kind='ExternalOutput')

    fe_ranges = _fe_split_ranges()
    AX = mybir.AxisListType
    ALU = mybir.AluOpType
    ACTF = mybir.ActivationFunctionType

    with TileContext(nc) as tc, ExitStack() as ctx:
        cpool = ctx.enter_context(tc.tile_pool(name='const', bufs=1))
        xpool = ctx.enter_context(tc.tile_pool(name='xin', bufs=33))
        fpool = ctx.enter_context(
            tc.tile_pool(name='fe_ps', bufs=1, space='PSUM'))
        spool = ctx.enter_context(
            tc.tile_pool(name='sp_ps', bufs=2, space='PSUM'))
        wpool = ctx.enter_context(tc.tile_pool(name='work', bufs=2))
        ppool = ctx.enter_context(tc.tile_pool(name='prod', bufs=4))
        opool = ctx.enter_context(tc.tile_pool(name='outb', bufs=1))

        def cload(dram, shape, tag, dtype=F16):
            t = cpool.tile(shape, dtype, tag=tag)
            nc.sync.dma_start(out=t[:], in_=dram[:])
            return t

        fw_sb = cload(fw_d, [44, 14 * NF], 'fw')
        albd_sb = cload(albd_d, [128, 112], 'albd')
        ngq_sb = cload(ngq_d, [P, 588], 'ngq')
        ngk_sb = cload(ngk_d, [P, 588], 'ngk')
        cqk_sb = cload(cqk_d, [P, 588], 'cqk')
        ngv_sb = cload(ngv_d, [P, 1344], 'ngv')
        cv_sb = cload(cv_d, [P, 1344], 'cv')
        id_sb = cload(id_d, [128, 128], 'ident')
        l2_sb = cload(l2_d, [33, OUT], 'l2')
        l47_sb = cload(l47_d, [101, 200], 'l47')
        l5_sb = cload(l5_d, [101, HID], 'l5')
        l8_sb = cload(l8_d, [101, HID], 'l8')
        l6_sb = cload(l6_d, [101, A1], 'l6')
        l9_sb = cload(l9_d, [101, A2], 'l9')
        neg_sb = cload(neg_d, [P, A1], 'neg', F32)
        mask_sb = cpool.tile([P, nt, A1], mybir.dt.uint8, tag='mask')
        nc.sync.dma_start(
            out=mask_sb[:], in_=mask_d[:].rearrange('(t p) c -> p t c', p=P))

        outb = opool.tile([P, nt, A1 + A2], F32, tag='outb')

        for it in range(nt):
            xg_t = xpool.tile([44, 4, P], F16, tag='xg')
            nc.sync.dma_start(out=xg_t[:],
                              in_=xg_d[:, :, it * P:(it + 1) * P])

            # ---- fused front-end -> fe psum [128, 14*216] (6 banks) ----
            fe = fpool.tile([P, 14 * NF], F32, tag='fe')
            for (t, lo, hi) in fe_ranges:
                g = t // 4
                kr = 44 if g < 3 else 22
                nc.tensor.matmul(
                    fe[:, t * NF + lo:t * NF + hi],
                    xg_t[0:kr, g, :],
                    fw_sb[0:kr, t * NF + lo:t * NF + hi],
                    start=True, stop=True)
            fe_t = fe[:].rearrange('p (t c) -> p t c', c=NF)

            # ---- LN stats for q/k/v ----
            msum = wpool.tile([P, 3], F32, tag='msum')
            nc.vector.tensor_reduce(
                msum[:], fe_t[:, :, 213:216].rearrange('p t c -> p c t'),
                AX.X, ALU.add)
            usq = wpool.tile([P, N, 33], F16, tag='usq')
            nc.scalar.activation(usq[:], fe_t[:, :, 180:213], ACTF.Square)
            ssq = wpool.tile([P, 3], F32, tag='ssq')
            nc.vector.tensor_reduce(
                ssq[:], usq[:].rearrange('p t (g i) -> p g t i', g=3),
                AX.XY, ALU.add)
            msq = wpool.tile([P, 3], F32, tag='msq')
            nc.vector.tensor_tensor(msq[:], msum[:], msum[:], ALU.mult)
            ssqn = wpool.tile([P, 3], F32, tag='ssqn')
            nc.scalar.activation(ssqn[:], ssq[:], ACTF.Copy,
                                 bias=EPS, scale=1.0 / HND)
            vv = wpool.tile([P, 3], F32, tag='vv')
            nc.vector.scalar_tensor_tensor(
                vv[:], msq[:], -1.0 / (HND * HND), ssqn[:],
                op0=ALU.mult, op1=ALU.add)
            vr = wpool.tile([P, 3], F32, tag='vr')
            nc.vector.reciprocal(vr[:], vv[:])
            rr = wpool.tile([P, 3], F32, tag='rr')
            nc.scalar.activation(rr[:], vr[:], ACTF.Sqrt)
            mt = wpool.tile([P, 3], F32, tag='mt')
            nc.vector.scalar_tensor_tensor(
                mt[:], msum[:], 1.0 / HND, rr[:], op0=ALU.mult, op1=ALU.mult)
            r_q, r_k, r_v = rr[:, 0:1], rr[:, 1:2], rr[:, 2:3]
            mt_q, mt_k, mt_v = mt[:, 0:1], mt[:, 1:2], mt[:, 2:3]

            # ---- pre-ELU scores (f,h,c)-major -> ELU -> at [p, G, 16] ----
            # G = f*H + h so the PSUM node-major (f, (h c)) view is 3D
            at = wpool.tile([P, 42, 16], F16, tag='at')
            nc.gpsimd.memset(at[:, :, 14:15], 1.0)
            nc.gpsimd.memset(at[:, :, 15:16], 0.0)
            feq = fe_t[:, :, 0:42]           # [p, f, (h c)]
            fek = fe_t[:, :, 42:84]
            asm1 = wpool.tile([P, 588], F16, tag='asm1')
            a3v = lambda t: t[:].rearrange('p (f x) -> p f x', x=42)
            nc.scalar.activation(a3v(asm1), feq, ACTF.Copy, scale=r_q)
            asm2 = wpool.tile([P, 588], F16, tag='asm2')
            nc.vector.scalar_tensor_tensor(
                a3v(asm2), fek, r_k, a3v(asm1), op0=ALU.mult, op1=ALU.add)
            asm3 = wpool.tile([P, 588], F16, tag='asm3')
            nc.vector.scalar_tensor_tensor(
                asm3[:], ngq_sb[:], mt_q, asm2[:],
                op0=ALU.mult, op1=ALU.add)
            asm4 = wpool.tile([P, 588], F16, tag='asm4')
            nc.vector.scalar_tensor_tensor(
                asm4[:], ngk_sb[:], mt_k, asm3[:],
                op0=ALU.mult, op1=ALU.add)
            apre = wpool.tile([P, 588], F16, tag='apre')
            nc.gpsimd.tensor_tensor(apre[:], asm4[:], cqk_sb[:], ALU.add)
            emn = wpool.tile([P, 588], F16, tag='emn')
            nc.gpsimd.tensor_scalar_min(emn[:], apre[:], 0.0)
            eex = wpool.tile([P, 588], F16, tag='eex')
            nc.scalar.activation(eex[:], emn[:], ACTF.Exp)
            erl = wpool.tile([P, 588], F16, tag='erl')
            nc.vector.tensor_relu(erl[:], apre[:])
            nc.vector.tensor_tensor(
                at[:, :, 0:14], eex[:].rearrange('p (g c) -> p g c', c=14),
                erl[:].rearrange('p (g c) -> p g c', c=14), ALU.add)

            # ---- transpose scores; alin matmul; exp ----
            xt = wpool.tile([P, 42, 14], F16, tag='xt')
            atf = at[:].rearrange('p g m -> p (g m)')
            for r in range(6):
                cw = 128 if r < 5 else 32
                gw = cw // 16
                tp = spool.tile([128, 256], F16, tag='sp')
                nc.tensor.transpose(
                    tp[0:cw, 0:128], atf[:, 128 * r:128 * r + cw], id_sb[:])
                att = wpool.tile([128, 128], F16, tag='att')
                if r % 2 == 0:
                    nc.scalar.copy(att[0:cw, :], tp[0:cw, 0:128])
                else:
                    nc.vector.tensor_copy(att[0:cw, :], tp[0:cw, 0:128])
                sp = spool.tile([128, 256], F32, tag='sp')
                nc.tensor.matmul(
                    sp[:, 0:14 * gw], att[0:cw, :],
                    albd_sb[0:cw, 0:14 * gw], start=True, stop=True)
                nc.scalar.activation(
                    xt[:, 8 * r:8 * r + gw, :],
                    sp[:, 0:14 * gw].rearrange('p (g c) -> p g c', c=14),
                    ACTF.Exp)

            # ---- softmax (P stays (f,h,c)-flat) ----
            zs = wpool.tile([P, 42], F32, tag='zs')
            nc.vector.tensor_reduce(zs[:], xt[:], AX.X, ALU.add)
            zr = wpool.tile([P, 42], F32, tag='zr')
            nc.vector.reciprocal(zr[:], zs[:])
            pt = wpool.tile([P, 42, 14], F16, tag='pt')
            nc.vector.tensor_tensor(
                pt[:], xt[:],
                zr[:].unsqueeze(-1).broadcast_to((P, 42, 14)),
                ALU.mult)

            # ---- V-path assembly into VL3 [p, j, (h c)] ----
            vl0 = wpool.tile([P, D, H * N], F16, tag='vl0')
            for h in range(H):
                # fe XV block for head h: [p, c, j] -> out [p, j, c]
                src = fe_t[:, :, 84 + D * h:84 + D * (h + 1)]
                dst = vl0[:, :, N * h:N * (h + 1)]
                if h == 0:
                    nc.scalar.activation(
                        dst.rearrange('p j c -> p c j'), src,
                        ACTF.Copy, scale=r_v)
                else:
                    nc.vector.tensor_scalar_mul(
                        dst.rearrange('p j c -> p c j'), src, r_v)
            vl1 = wpool.tile([P, D * H * N], F16, tag='vl1')
            nc.vector.scalar_tensor_tensor(
                vl1[:], ngv_sb[:], mt_v,
                vl0[:].rearrange('p j x -> p (j x)'),
                op0=ALU.mult, op1=ALU.add)
            vl = wpool.tile([P, D, H * N], F16, tag='vl')
            nc.gpsimd.tensor_tensor(
                vl[:].rearrange('p j x -> p (j x)'), vl1[:], cv_sb[:],
                ALU.add)

            # ---- bilinear E = P @ VL: per-j products + strided reduces ----
            ttf = wpool.tile([P, N, H, D], F32, tag='ttf')
            for j in range(D):
                pj = ppool.tile([P, 588], F16, tag='prodj')
                eng = nc.gpsimd if (j % 8) < 3 else nc.vector
                eng.tensor_tensor(
                    pj[:].rearrange('p (f x) -> p f x', x=42),
                    pt[:].rearrange('p (f h) c -> p f (h c)', h=H),
                    vl[:, j, :].unsqueeze(1).broadcast_to((P, N, 42)),
                    ALU.mult)
                nc.vector.tensor_reduce(
                    ttf[:, :, :, j],
                    pj[:].rearrange('p (f h c) -> p (f h) c', h=H, c=14),
                    AX.X, ALU.add)

            # ---- head-sum, relu, LN2 + maxpool ----
            l1a = wpool.tile([P, N * D], F32, tag='l1a')
            nc.vector.tensor_tensor(
                l1a[:].rearrange('p (f j) -> p f j', f=N),
                ttf[:, :, 0, :], ttf[:, :, 1, :], ALU.add)
            l1b = wpool.tile([P, N * D], F32, tag='l1b')
            nc.vector.tensor_tensor(
                l1b[:].rearrange('p (f j) -> p f j', f=N),
                l1a[:].rearrange('p (f j) -> p f j', f=N),
                ttf[:, :, 2, :], ALU.add)
            rl1 = wpool.tile([P, N * D], F16, tag='rl1')
            nc.vector.tensor_relu(rl1[:], l1b[:])
            m2s = wpool.tile([P, 1], F32, tag='m2s')
            nc.vector.tensor_reduce(m2s[:], rl1[:], AX.X, ALU.add)
            m2m = wpool.tile([P, 1], F32, tag='m2m')
            nc.scalar.activation(m2m[:], m2s[:], ACTF.Copy,
                                 scale=1.0 / (N * D))
            scr2 = wpool.tile([P, N * D], F16, tag='scr2')
            nc.scalar.activation(scr2[:], rl1[:], ACTF.Square)
            ss2 = wpool.tile([P, 1], F32, tag='ss2')
            nc.vector.tensor_reduce(ss2[:], scr2[:], AX.X, ALU.add)
            ms2 = wpool.tile([P, 1], F32, tag='ms2')
            nc.vector.tensor_tensor(ms2[:], m2m[:], m2m[:], ALU.mult)
            ssn2 = wpool.tile([P, 1], F32, tag='ssn2')
            nc.scalar.activation(ssn2[:], ss2[:], ACTF.Copy,
                                 bias=EPS, scale=1.0 / (N * D))
            vv2 = wpool.tile([P, 1], F32, tag='vv2')
            nc.vector.scalar_tensor_tensor(
                vv2[:], ms2[:], -1.0, ssn2[:], op0=ALU.mult, op1=ALU.add)
            vr2 = wpool.tile([P, 1], F32, tag='vr2')
            nc.vector.reciprocal(vr2[:], vv2[:])
            r2 = wpool.tile([P, 1], F32, tag='r2')
            nc.scalar.activation(r2[:], vr2[:], ACTF.Sqrt)
            mp = wpool.tile([P, D], F32, tag='mp')
            nc.vector.tensor_reduce(
                mp[:], rl1[:].rearrange('p (f j) -> p j f', f=N),
                AX.X, ALU.max)
            # ones-column carried through the transpose supplies the
            # bias row of each tail lhsT (partition starts must be 32-aligned)
            en = wpool.tile([P, D + 1], F16, tag='en')
            nc.vector.tensor_scalar(
                en[:, 0:D], mp[:], m2m[:], r2[:],
                op0=ALU.subtract, op1=ALU.mult)
            nc.gpsimd.memset(en[:, D:D + 1], 1.0)

            # ---- tail MLP (biases via ones-rows from ones-cols) ----
            def transp_aug(src_ap, rows, tag):
                tps = spool.tile([128, 256], F16, tag='sp')
                nc.tensor.transpose(tps[0:rows + 1, 0:128], src_ap, id_sb[:])
                agt = wpool.tile([rows + 1, P], F16, tag=tag)
                nc.scalar.copy(agt[:], tps[0:rows + 1, 0:128])
                return agt

            enT = transp_aug(en[:], D, 'enT')
            yp = spool.tile([128, 256], F32, tag='sp')
            nc.tensor.matmul(yp[:, 0:OUT], enT[:], l2_sb[:],
                             start=True, stop=True)
            ymn = wpool.tile([P, OUT], F16, tag='ymn')
            nc.vector.tensor_scalar_min(ymn[:], yp[:, 0:OUT], 0.0)
            yex = wpool.tile([P, OUT], F16, tag='yex')
            nc.scalar.activation(yex[:], ymn[:], ACTF.Exp)
            yrl = wpool.tile([P, OUT], F16, tag='yrl')
            nc.vector.tensor_relu(yrl[:], yp[:, 0:OUT])
            ye = wpool.tile([P, OUT + 1], F16, tag='ye')
            nc.vector.tensor_tensor(ye[:, 0:OUT], yex[:], yrl[:], ALU.add)
            nc.gpsimd.memset(ye[:, OUT:OUT + 1], 1.0)

            yeT = transp_aug(ye[:], OUT, 'yeT')
            a37p = spool.tile([128, 256], F32, tag='sp')
            nc.tensor.matmul(a37p[:, 0:200], yeT[:], l47_sb[:],
                             start=True, stop=True)
            t3 = wpool.tile([P, HID + 1], F16, tag='t3')
            nc.scalar.activation(t3[:, 0:HID], a37p[:, 0:HID], ACTF.Tanh)
            nc.gpsimd.memset(t3[:, HID:HID + 1], 1.0)
            t6 = wpool.tile([P, HID + 1], F16, tag='t6')
            nc.scalar.activation(t6[:, 0:HID], a37p[:, HID:200], ACTF.Tanh)
            nc.gpsimd.memset(t6[:, HID:HID + 1], 1.0)

            a3T = transp_aug(t3[:], HID, 'a3T')
            a6T = transp_aug(t6[:], HID, 'a6T')
            a4p = spool.tile([128, 256], F32, tag='sp')
            nc.tensor.matmul(a4p[:, 0:HID], a3T[:], l5_sb[:],
                             start=True, stop=True)
            a4 = wpool.tile([P, HID + 1], F16, tag='a4')
            nc.scalar.activation(a4[:, 0:HID], a4p[:, 0:HID], ACTF.Tanh)
            nc.gpsimd.memset(a4[:, HID:HID + 1], 1.0)
            a7p = spool.tile([128, 256], F32, tag='sp')
            nc.tensor.matmul(a7p[:, 0:HID], a6T[:], l8_sb[:],
                             start=True, stop=True)
            a7 = wpool.tile([P, HID + 1], F16, tag='a7')
            nc.scalar.activation(a7[:, 0:HID], a7p[:, 0:HID], ACTF.Tanh)
            nc.gpsimd.memset(a7[:, HID:HID + 1], 1.0)

            a4T = transp_aug(a4[:], HID, 'a4T')
            a7T = transp_aug(a7[:], HID, 'a7T')
            p1 = spool.tile([128, 256], F32, tag='sp')
            nc.tensor.matmul(p1[:, 0:A1], a4T[:], l6_sb[:],
                             start=True, stop=True)
            p2 = spool.tile([128, 256], F32, tag='sp')
            nc.tensor.matmul(p2[:, 0:A2], a7T[:], l9_sb[:],
                             start=True, stop=True)
            a5 = wpool.tile([P, A1], F32, tag='a5')
            nc.scalar.activation(a5[:], p1[:, 0:A1], ACTF.Tanh)
            nc.scalar.activation(outb[:, it, A1:A1 + A2], p2[:, 0:A2],
                                 ACTF.Sigmoid)
            nc.vector.select(outb[:, it, 0:A1], mask_sb[:, it, :],
                             a5[:], neg_sb[:])

        outb2 = opool.tile([P, nt, A1 + A2], F32, tag='outb2')
        nc.scalar.copy(outb2[:], outb[:])
        nc.sync.dma_start(
            out=out1_d[:].rearrange('(t p) c -> p t c', p=P),
            in_=outb2[:, :, 0:A1])
        nc.sync.dma_start(
            out=out2_d[:].rearrange('(t p) c -> p t c', p=P),
            in_=outb2[:, :, A1:A1 + A2])

    nc.compile()
    return nc


def kernel(**inputs):
    in_maps = _host_prep(inputs)
    nc = build_program(BC)
    res = run_bass_kernel_spmd(nc, in_maps, list(range(NCORES))).results
    out1 = np.concatenate([r['out1'] for r in res], axis=0)
    out2 = np.concatenate([r['out2'] for r in res], axis=0)
    return out1, out2


# revision 22
# speedup vs baseline: 470.5657x; 470.5657x over previous
"""Trainium2 Bass kernel for nn_Attention_module_actor (dense_transformer).

8-way data parallel (4096 samples/core, 32 tiles of 128 samples on SBUF
partitions). The QKV projection + LayerNorm(H,N,D) + additive-attention
linears are algebraically fused host-side into one per-node linear map of
the augmented input [x_node, 1]; LayerNorms reduce to per-sample scalars
(r = rsqrt(var), mtil = mean*r) obtained from Cholesky-factor projections.
PE does the fused front-end, score transposes + alin, and the dense tail
(biases folded via ones-rows); ACT applies per-partition-scalar LN scaling
and transcendentals; DVE/GPSIMD do softmax, the per-sample bilinear P@V
(broadcast products + strided reduce), LN2+maxpool (max commutes with the
positive affine), and the output mask select.
"""

import numpy as np
from contextlib import ExitStack

import concourse.bass as bass
import concourse.bacc as bacc
import concourse.mybir as mybir
from concourse.tile import TileContext
from concourse.bass_utils import run_bass_kernel_spmd

B, N, F_IN, H, D = 32768, 14, 10, 3, 32
HID, OUT, A1, A2 = 100, 100, 9, 8
EPS = 1e-5
NCORES = 8
BC = B // NCORES
P = 128
HND = H * N * D
E1 = F_IN + 1
NF = 216

F16 = mybir.dt.float16
F32 = mybir.dt.float32


# ---------------------------------------------------------------------------
# host-side weight fusion
# ---------------------------------------------------------------------------

def _fuse_qk(p_w, p_b, g, bb, lw, lb):
    pw = p_w.reshape(F_IN, H, D)
    pb = p_b.reshape(H, D)
    Wt = np.einsum('ehd,htd,dc->ehtc', pw, g, lw)
    Wb = np.einsum('hd,htd,dc->htc', pb, g, lw)
    W = np.concatenate([Wt, Wb[None]], axis=0)
    NG = -np.einsum('htd,dc->htc', g, lw)
    CONST = np.einsum('htd,dc->htc', bb, lw) + lb[None, None, :]
    return W, NG, CONST


def _fuse_v(v_w, v_b, g, bb, l1w, l1b):
    vw = v_w.reshape(F_IN, H, D)
    vb = v_b.reshape(H, D)
    l1 = l1w.reshape(H, D, D)
    Wt = np.einsum('ehd,hcd,hdj->ehcj', vw, g, l1)
    Wb = np.einsum('hd,hcd,hdj->hcj', vb, g, l1)
    W = np.concatenate([Wt, Wb[None]], axis=0)
    NG = -np.einsum('hcd,hdj->hcj', g, l1)
    # softmax rows sum to 1 -> lin1_b/H per (h,c) sums back to lin1_b
    CONST = np.einsum('hcd,hdj->hcj', bb, l1) + l1b[None, None, :] / H
    return W, NG, CONST


def _stats_cols(p_w, p_b):
    Wbar = np.concatenate([p_w, p_b[None, :]], axis=0)
    Gram = Wbar @ Wbar.T
    reg = 1e-9 * (np.trace(Gram) / E1 + 1.0)
    L = np.linalg.cholesky(Gram + reg * np.eye(E1))
    M = Wbar.sum(axis=1)
    return L, M


def _host_prep(inputs):
    f = {k: np.asarray(v, np.float64) for k, v in inputs.items()
         if k != 'mask_vec'}
    mask = np.asarray(inputs['mask_vec'])

    Wq, NGq, Cq = _fuse_qk(f['q_w'], f['q_b'], f['qn_g'], f['qn_b'],
                           f['qlin_w'], f['qlin_b'])
    Wk, NGk, Ck = _fuse_qk(f['k_w'], f['k_b'], f['kn_g'], f['kn_b'],
                           f['klin_w'], f['klin_b'])
    Wv, NGv, Cv = _fuse_v(f['v_w'], f['v_b'], f['vn_g'], f['vn_b'],
                          f['lin1_w'], f['lin1_b'])
    Lq, Mq = _stats_cols(f['q_w'], f['q_b'])
    Lk, Mk = _stats_cols(f['k_w'], f['k_b'])
    Lv, Mv = _stats_cols(f['v_w'], f['v_b'])

    fw = np.zeros((44, 14 * NF), np.float64)
    for t in range(N):
        i = t % 4
        blk = np.zeros((E1, NF))
        blk[:, 0:42] = Wq[:, :, t, :].reshape(E1, 42)
        blk[:, 42:84] = Wk[:, :, t, :].reshape(E1, 42)
        blk[:, 84:180] = Wv[:, :, t, :].reshape(E1, 96)
        blk[:, 180:191] = Lq
        blk[:, 191:202] = Lk
        blk[:, 202:213] = Lv
        blk[:, 213] = Mq
        blk[:, 214] = Mk
        blk[:, 215] = Mv
        fw[11 * i:11 * i + 11, t * NF:(t + 1) * NF] = blk

    alin_w = f['alin_w']
    alin_bp = f['alin_b'] - alin_w.sum(axis=0)   # fold ELU's -1 shift
    albd = np.zeros((128, 112), np.float64)
    for g in range(8):
        albd[16 * g:16 * g + 14, 14 * g:14 * g + 14] = alin_w
        albd[16 * g + 14, 14 * g:14 * g + 14] = alin_bp

    rep = lambda a: np.broadcast_to(
        a.reshape(1, -1), (P, a.size)).astype(np.float16).copy()
    aug = lambda w, bvec: np.concatenate(
        [w, bvec[None, :]], axis=0).astype(np.float16)

    consts = {
        'fw': fw.astype(np.float16),
        'albd': albd.astype(np.float16),
        'ngq': rep(NGq.transpose(1, 0, 2)),
        'ngk': rep(NGk.transpose(1, 0, 2)),
        'cqk': rep((Cq + Ck).transpose(1, 0, 2)),
        'ngv': rep(NGv.transpose(2, 0, 1)),
        'cv': rep(Cv.transpose(2, 0, 1)),
        'ident': np.eye(128).astype(np.float16),
        'lin2a': aug(f['lin2_w'], f['lin2_b']),
        'l47a': aug(np.concatenate([f['l4_w'], f['l7_w']], 1),
                    np.concatenate([f['l4_b'] - f['l4_w'].sum(0),
                                    f['l7_b'] - f['l7_w'].sum(0)])),
        'l5a': aug(f['l5_w'], f['l5_b']),
        'l8a': aug(f['l8_w'], f['l8_b']),
        'l6a': aug(f['l6_w'], f['l6_b']),
        'l9a': aug(f['l9_w'], f['l9_b']),
        'negrep': np.full((P, A1), -1e8, np.float32),
    }

    x = np.asarray(inputs['x'], np.float32)
    in_maps = []
    for c in range(NCORES):
        xs = x[c * BC:(c + 1) * BC]
        xg = np.zeros((44, 4, BC), np.float16)
        for t in range(N):
            g, i = t // 4, t % 4
            xg[11 * i:11 * i + 10, g, :] = xs[:, t, :].T.astype(np.float16)
            xg[11 * i + 10, g, :] = 1.0
        im = {'xg': xg, 'mask': mask[c * BC:(c + 1) * BC].astype(np.uint8)}
        im.update(consts)
        in_maps.append(im)
    return in_maps


# ---------------------------------------------------------------------------
# device program
# ---------------------------------------------------------------------------

def _fe_split_ranges():
    """FE matmul col ranges split so no PSUM output crosses a 2KB bank."""
    out = []
    for t in range(N):
        b0, b1 = t * NF * 4, (t + 1) * NF * 4
        rngs = [(b0, b1)]
        if b0 // 2048 != (b1 - 1) // 2048:
            edge = ((b0 // 2048) + 1) * 2048
            rngs = [(b0, edge), (edge, b1)]
        for lo, hi in rngs:
            out.append((t, (lo - b0) // 4, (hi - b0) // 4))
    return out


def build_program(bc=BC, repeat=1):
    nt = bc // P
    nc = bacc.Bacc()

    xg_d = nc.dram_tensor('xg', [44, 4, bc], F16, kind='ExternalInput')
    mask_d = nc.dram_tensor('mask', [bc, A1], mybir.dt.uint8, kind='ExternalInput')
    fw_d = nc.dram_tensor('fw', [44, 14 * NF], F16, kind='ExternalInput')
    albd_d = nc.dram_tensor('albd', [128, 112], F16, kind='ExternalInput')
    ngq_d = nc.dram_tensor('ngq', [P, 588], F16, kind='ExternalInput')
    ngk_d = nc.dram_tensor('ngk', [P, 588], F16, kind='ExternalInput')
    cqk_d = nc.dram_tensor('cqk', [P, 588], F16, kind='ExternalInput')
    ngv_d = nc.dram_tensor('ngv', [P, 1344], F16, kind='ExternalInput')
    cv_d = nc.dram_tensor('cv', [P, 1344], F16, kind='ExternalInput')
    id_d = nc.dram_tensor('ident', [128, 128], F16, kind='ExternalInput')
    l2_d = nc.dram_tensor('lin2a', [33, OUT], F16, kind='ExternalInput')
    l47_d = nc.dram_tensor('l47a', [101, 200], F16, kind='ExternalInput')
    l5_d = nc.dram_tensor('l5a', [101, HID], F16, kind='ExternalInput')
    l8_d = nc.dram_tensor('l8a', [101, HID], F16, kind='ExternalInput')
    l6_d = nc.dram_tensor('l6a', [101, A1], F16, kind='ExternalInput')
    l9_d = nc.dram_tensor('l9a', [101, A2], F16, kind='ExternalInput')
    neg_d = nc.dram_tensor('negrep', [P, A1], F32, kind='ExternalInput')
    out1_d = nc.dram_tensor('out1', [bc, A1], F32, kind='ExternalOutput')
    out2_d = nc.dram_tensor('out2', [bc, A2], F32, kind='ExternalOutput')

    fe_ranges = _fe_split_ranges()
    AX = mybir.AxisListType
    ALU = mybir.AluOpType
    ACTF = mybir.ActivationFunctionType

    with nc.allow_low_precision('f16 partial sums verified vs reference'), \
            TileContext(nc) as tc, ExitStack() as ctx:
        cpool = ctx.enter_context(tc.tile_pool(name='const', bufs=1))
        xpool = ctx.enter_context(tc.tile_pool(name='xin', bufs=33))
        fpool = ctx.enter_context(
            tc.tile_pool(name='fe_ps', bufs=1, space='PSUM'))
        spool = ctx.enter_context(
            tc.tile_pool(name='sp_ps', bufs=2, space='PSUM'))
        wpool = ctx.enter_context(tc.tile_pool(name='work', bufs=2))
        ppool = ctx.enter_context(tc.tile_pool(name='prod', bufs=4))
        opool = ctx.enter_context(tc.tile_pool(name='outb', bufs=1))

        def cload(dram, shape, tag, dtype=F16):
            t = cpool.tile(shape, dtype, tag=tag)
            nc.sync.dma_start(out=t[:], in_=dram[:])
            return t

        fw_sb = cload(fw_d, [44, 14 * NF], 'fw')
        albd_sb = cload(albd_d, [128, 112], 'albd')
        ngq_sb = cload(ngq_d, [P, 588], 'ngq')
        ngk_sb = cload(ngk_d, [P, 588], 'ngk')
        cqk_sb = cload(cqk_d, [P, 588], 'cqk')
        ngv_sb = cload(ngv_d, [P, 1344], 'ngv')
        cv_sb = cload(cv_d, [P, 1344], 'cv')
        id_sb = cload(id_d, [128, 128], 'ident')
        l2_sb = cload(l2_d, [33, OUT], 'l2')
        l47_sb = cload(l47_d, [101, 200], 'l47')
        l5_sb = cload(l5_d, [101, HID], 'l5')
        l8_sb = cload(l8_d, [101, HID], 'l8')
        l6_sb = cload(l6_d, [101, A1], 'l6')
        l9_sb = cload(l9_d, [101, A2], 'l9')
        neg_sb = cload(neg_d, [P, A1], 'neg', F32)
        mask_sb = cpool.tile([P, nt, A1], mybir.dt.uint8, tag='mask')
        nc.sync.dma_start(
            out=mask_sb[:], in_=mask_d[:].rearrange('(t p) c -> p t c', p=P))

        outb = opool.tile([P, nt, A1 + A2], F32, tag='outb')

        rep_ctx = tc.For_i(0, repeat, 1) if repeat > 1 else None
        if rep_ctx is not None:
            rep_ctx.__enter__()
        for it in range(nt):
            xg_t = xpool.tile([44, 4, P], F16, tag='xg')
            nc.sync.dma_start(out=xg_t[:],
                              in_=xg_d[:, :, it * P:(it + 1) * P])

            # ---- fused front-end -> fe psum [128, 14*216] (6 banks) ----
            fe = fpool.tile([P, 14 * NF], F32, tag='fe')
            for (t, lo, hi) in fe_ranges:
                g = t // 4
                kr = 44 if g < 3 else 22
                nc.tensor.matmul(
                    fe[:, t * NF + lo:t * NF + hi],
                    xg_t[0:kr, g, :],
                    fw_sb[0:kr, t * NF + lo:t * NF + hi],
                    start=True, stop=True)
            fe_t = fe[:].rearrange('p (t c) -> p t c', c=NF)

            # ---- LN stats for q/k/v ----
            msum = wpool.tile([P, 3], F32, tag='msum')
            nc.vector.tensor_reduce(
                msum[:], fe_t[:, :, 213:216].rearrange('p t c -> p c t'),
                AX.X, ALU.add)
            uc = wpool.tile([P, N, 33], F16, tag='uc')
            nc.scalar.activation(uc[:], fe_t[:, :, 180:213], ACTF.Copy)
            usq = wpool.tile([P, N, 33], F16, tag='usq')
            nc.gpsimd.tensor_tensor(usq[:], uc[:], uc[:], ALU.mult)
            ssq = wpool.tile([P, 3], F32, tag='ssq')
            nc.vector.tensor_reduce(
                ssq[:], usq[:].rearrange('p t (g i) -> p g t i', g=3),
                AX.XY, ALU.add)
            msq = wpool.tile([P, 3], F32, tag='msq')
            nc.vector.tensor_tensor(msq[:], msum[:], msum[:], ALU.mult)
            ssqn = wpool.tile([P, 3], F32, tag='ssqn')
            nc.scalar.activation(ssqn[:], ssq[:], ACTF.Copy,
                                 bias=EPS, scale=1.0 / HND)
            vv = wpool.tile([P, 3], F32, tag='vv')
            nc.vector.scalar_tensor_tensor(
                vv[:], msq[:], -1.0 / (HND * HND), ssqn[:],
                op0=ALU.mult, op1=ALU.add)
            vl_ = wpool.tile([P, 3], F32, tag='vl_')
            nc.scalar.activation(vl_[:], vv[:], ACTF.Ln)
            rr = wpool.tile([P, 3], F32, tag='rr')
            nc.scalar.activation(rr[:], vl_[:], ACTF.Exp, scale=-0.5)
            mt = wpool.tile([P, 3], F32, tag='mt')
            nc.vector.scalar_tensor_tensor(
                mt[:], msum[:], 1.0 / HND, rr[:], op0=ALU.mult, op1=ALU.mult)
            r_q, r_k, r_v = rr[:, 0:1], rr[:, 1:2], rr[:, 2:3]
            mt_q, mt_k, mt_v = mt[:, 0:1], mt[:, 1:2], mt[:, 2:3]

            # ---- pre-ELU scores (f,h,c)-major -> ELU -> at [p, G, 16] ----
            # G = f*H + h so the PSUM node-major (f, (h c)) view is 3D
            at = wpool.tile([P, 42, 16], F16, tag='at')
            nc.gpsimd.memset(at[:, :, 14:15], 1.0)
            nc.gpsimd.memset(at[:, :, 15:16], 0.0)
            feq = fe_t[:, :, 0:42]           # [p, f, (h c)]
            fek = fe_t[:, :, 42:84]
            asm1 = wpool.tile([P, 588], F16, tag='asm1')
            a3v = lambda t: t[:].rearrange('p (f x) -> p f x', x=42)
            nc.scalar.activation(a3v(asm1), feq, ACTF.Copy, scale=r_q)
            asm2 = wpool.tile([P, 588], F16, tag='asm2')
            nc.vector.scalar_tensor_tensor(
                a3v(asm2), fek, r_k, a3v(asm1), op0=ALU.mult, op1=ALU.add)
            asm3 = wpool.tile([P, 588], F16, tag='asm3')
            nc.vector.scalar_tensor_tensor(
                asm3[:], ngq_sb[:], mt_q, asm2[:],
                op0=ALU.mult, op1=ALU.add)
            asm4 = wpool.tile([P, 588], F16, tag='asm4')
            nc.vector.scalar_tensor_tensor(
                asm4[:], ngk_sb[:], mt_k, asm3[:],
                op0=ALU.mult, op1=ALU.add)
            apre = wpool.tile([P, 588], F16, tag='apre')
            nc.gpsimd.tensor_tensor(apre[:], asm4[:], cqk_sb[:], ALU.add)
            emn = wpool.tile([P, 588], F16, tag='emn')
            nc.gpsimd.tensor_scalar_min(emn[:], apre[:], 0.0)
            eex = wpool.tile([P, 588], F16, tag='eex')
            nc.scalar.activation(eex[:], emn[:], ACTF.Exp)
            erl = wpool.tile([P, 588], F16, tag='erl')
            nc.vector.tensor_relu(erl[:], apre[:])
            nc.vector.tensor_tensor(
                at[:, :, 0:14], eex[:].rearrange('p (g c) -> p g c', c=14),
                erl[:].rearrange('p (g c) -> p g c', c=14), ALU.add)

            # ---- transpose scores; alin matmul; exp ----
            xt = wpool.tile([P, 42, 14], F16, tag='xt')
            atf = at[:].rearrange('p g m -> p (g m)')
            for r in range(6):
                cw = 128 if r < 5 else 32
                gw = cw // 16
                tp = spool.tile([128, 256], F16, tag='sp')
                nc.tensor.transpose(
                    tp[0:cw, 0:128], atf[:, 128 * r:128 * r + cw], id_sb[:])
                att = wpool.tile([128, 128], F16, tag='att')
                if r % 2 == 0:
                    nc.scalar.copy(att[0:cw, :], tp[0:cw, 0:128])
                else:
                    nc.vector.tensor_copy(att[0:cw, :], tp[0:cw, 0:128])
                sp = spool.tile([128, 256], F32, tag='sp')
                nc.tensor.matmul(
                    sp[:, 0:14 * gw], att[0:cw, :],
                    albd_sb[0:cw, 0:14 * gw], start=True, stop=True)
                nc.scalar.activation(
                    xt[:, 8 * r:8 * r + gw, :],
                    sp[:, 0:14 * gw].rearrange('p (g c) -> p g c', c=14),
                    ACTF.Exp)

            # ---- softmax (P stays (f,h,c)-flat) ----
            zs = wpool.tile([P, 42], F32, tag='zs')
            nc.vector.tensor_reduce(zs[:], xt[:], AX.X, ALU.add)
            zr = wpool.tile([P, 42], F32, tag='zr')
            nc.vector.reciprocal(zr[:], zs[:])
            pt = wpool.tile([P, 42, 14], F16, tag='pt')
            nc.vector.tensor_tensor(
                pt[:], xt[:],
                zr[:].unsqueeze(-1).broadcast_to((P, 42, 14)),
                ALU.mult)

            # ---- V-path assembly into VL3 [p, j, (h c)] ----
            vl0 = wpool.tile([P, D, H * N], F16, tag='vl0')
            for h in range(H):
                # fe XV block for head h: [p, c, j] -> out [p, j, c]
                src = fe_t[:, :, 84 + D * h:84 + D * (h + 1)]
                dst = vl0[:, :, N * h:N * (h + 1)]
                if h == 0:
                    nc.scalar.activation(
                        dst.rearrange('p j c -> p c j'), src,
                        ACTF.Copy, scale=r_v)
                else:
                    nc.vector.tensor_scalar_mul(
                        dst.rearrange('p j c -> p c j'), src, r_v)
            vl1 = wpool.tile([P, D * H * N], F16, tag='vl1')
            nc.vector.scalar_tensor_tensor(
                vl1[:], ngv_sb[:], mt_v,
                vl0[:].rearrange('p j x -> p (j x)'),
                op0=ALU.mult, op1=ALU.add)
            vl = wpool.tile([P, D, H * N], F16, tag='vl')
            nc.gpsimd.tensor_tensor(
                vl[:].rearrange('p j x -> p (j x)'), vl1[:], cv_sb[:],
                ALU.add)

            # ---- bilinear E = P @ VL: j-blocked products + reduces ----
            # ttf [p, j, f, h] f16 keeps every reduce output packed-last
            ttf = wpool.tile([P, D, N, H], F16, tag='ttf')
            JB = 4
            pv3 = pt[:].rearrange('p (f h) c -> p f (h c)', h=H)
            for j0 in range(0, D, JB):
                if (j0 // JB) % 4 == 3:
                    # spread some products to GPSIMD singly
                    for j in range(j0, j0 + JB):
                        pj = ppool.tile([P, 588], F16, tag='prodj')
                        nc.gpsimd.tensor_tensor(
                            pj[:].rearrange('p (f x) -> p f x', x=42),
                            pv3,
                            vl[:, j, :].unsqueeze(1)
                              .broadcast_to((P, N, 42)),
                            ALU.mult)
                        nc.vector.tensor_reduce(
                            ttf[:, j, :, :],
                            pj[:].rearrange(
                                'p (f h c) -> p (f h) c', h=H, c=14),
                            AX.X, ALU.add)
                else:
                    pj4 = ppool.tile([P, JB, N, 42], F16, tag='prodj4')
                    nc.vector.tensor_tensor(
                        pj4[:],
                        pt[:].rearrange('p (f h) c -> p f (h c)', h=H)
                            .unsqueeze(1).broadcast_to((P, JB, N, 42)),
                        vl[:, j0:j0 + JB, :].unsqueeze(2)
                          .broadcast_to((P, JB, N, 42)),
                        ALU.mult)
                    nc.vector.tensor_reduce(
                        ttf[:, j0:j0 + JB, :, :],
                        pj4[:].rearrange(
                            'p j f (h c) -> p (j f h) c', c=14),
                        AX.X, ALU.add)

            # ---- head-sum (one packed reduce), relu ----
            rl0 = wpool.tile([P, D * N], F16, tag='rl0')
            nc.vector.tensor_reduce(
                rl0[:], ttf[:].rearrange('p j f h -> p (j f) h'),
                AX.X, ALU.add)
            rl1 = wpool.tile([P, D * N], F16, tag='rl1')
            nc.vector.tensor_relu(rl1[:], rl0[:])
            m2s = wpool.tile([P, 1], F32, tag='m2s')
            nc.vector.tensor_reduce(m2s[:], rl1[:], AX.X, ALU.add)
            m2m = wpool.tile([P, 1], F32, tag='m2m')
            nc.scalar.activation(m2m[:], m2s[:], ACTF.Copy,
                                 scale=1.0 / (N * D))
            scr2 = wpool.tile([P, D * N], F16, tag='scr2')
            nc.gpsimd.tensor_tensor(scr2[:], rl1[:], rl1[:], ALU.mult)
            ss2 = wpool.tile([P, 1], F32, tag='ss2')
            nc.vector.tensor_reduce(ss2[:], scr2[:], AX.X, ALU.add)
            ms2 = wpool.tile([P, 1], F32, tag='ms2')
            nc.vector.tensor_tensor(ms2[:], m2m[:], m2m[:], ALU.mult)
            ssn2 = wpool.tile([P, 1], F32, tag='ssn2')
            nc.scalar.activation(ssn2[:], ss2[:], ACTF.Copy,
                                 bias=EPS, scale=1.0 / (N * D))
            vv2 = wpool.tile([P, 1], F32, tag='vv2')
            nc.vector.scalar_tensor_tensor(
                vv2[:], ms2[:], -1.0, ssn2[:], op0=ALU.mult, op1=ALU.add)
            vl2_ = wpool.tile([P, 1], F32, tag='vl2_')
            nc.scalar.activation(vl2_[:], vv2[:], ACTF.Ln)
            r2 = wpool.tile([P, 1], F32, tag='r2')
            nc.scalar.activation(r2[:], vl2_[:], ACTF.Exp, scale=-0.5)
            mp = wpool.tile([P, D], F16, tag='mp')
            nc.vector.tensor_reduce(
                mp[:], rl1[:].rearrange('p (j f) -> p j f', f=N),
                AX.X, ALU.max)
            # ones-column carried through the transpose supplies the
            # bias row of each tail lhsT (partition starts must be 32-aligned)
            en = wpool.tile([P, D + 1], F16, tag='en')
            nc.vector.tensor_scalar(
                en[:, 0:D], mp[:], m2m[:], r2[:],
                op0=ALU.subtract, op1=ALU.mult)
            nc.gpsimd.memset(en[:, D:D + 1], 1.0)

            # ---- tail MLP (biases via ones-rows from ones-cols) ----
            def transp_aug(src_ap, rows, tag):
                tps = spool.tile([128, 256], F16, tag='sp')
                nc.tensor.transpose(tps[0:rows + 1, 0:128], src_ap, id_sb[:])
                agt = wpool.tile([rows + 1, P], F16, tag=tag)
                nc.scalar.copy(agt[:], tps[0:rows + 1, 0:128])
                return agt

            enT = transp_aug(en[:], D, 'enT')
            yp = spool.tile([128, 256], F32, tag='sp')
            nc.tensor.matmul(yp[:, 0:OUT], enT[:], l2_sb[:],
                             start=True, stop=True)
            ymn = wpool.tile([P, OUT], F16, tag='ymn')
            nc.vector.tensor_scalar_min(ymn[:], yp[:, 0:OUT], 0.0)
            yex = wpool.tile([P, OUT], F16, tag='yex')
            nc.scalar.activation(yex[:], ymn[:], ACTF.Exp)
            yrl = wpool.tile([P, OUT], F16, tag='yrl')
            nc.vector.tensor_relu(yrl[:], yp[:, 0:OUT])
            ye = wpool.tile([P, OUT + 1], F16, tag='ye')
            nc.vector.tensor_tensor(ye[:, 0:OUT], yex[:], yrl[:], ALU.add)
            nc.gpsimd.memset(ye[:, OUT:OUT + 1], 1.0)

            yeT = transp_aug(ye[:], OUT, 'yeT')
            a37p = spool.tile([128, 256], F32, tag='sp')
            nc.tensor.matmul(a37p[:, 0:200], yeT[:], l47_sb[:],
                             start=True, stop=True)
            t3 = wpool.tile([P, HID + 1], F16, tag='t3')
            nc.scalar.activation(t3[:, 0:HID], a37p[:, 0:HID], ACTF.Tanh)
            nc.gpsimd.memset(t3[:, HID:HID + 1], 1.0)
            t6 = wpool.tile([P, HID + 1], F16, tag='t6')
            nc.scalar.activation(t6[:, 0:HID], a37p[:, HID:200], ACTF.Tanh)
            nc.gpsimd.memset(t6[:, HID:HID + 1], 1.0)

            a3T = transp_aug(t3[:], HID, 'a3T')
            a6T = transp_aug(t6[:], HID, 'a6T')
            a4p = spool.tile([128, 256], F32, tag='sp')
            nc.tensor.matmul(a4p[:, 0:HID], a3T[:], l5_sb[:],
                             start=True, stop=True)
            a4 = wpool.tile([P, HID + 1], F16, tag='a4')
            nc.scalar.activation(a4[:, 0:HID], a4p[:, 0:HID], ACTF.Tanh)
            nc.gpsimd.memset(a4[:, HID:HID + 1], 1.0)
            a7p = spool.tile([128, 256], F32, tag='sp')
            nc.tensor.matmul(a7p[:, 0:HID], a6T[:], l8_sb[:],
                             start=True, stop=True)
            a7 = wpool.tile([P, HID + 1], F16, tag='a7')
            nc.scalar.activation(a7[:, 0:HID], a7p[:, 0:HID], ACTF.Tanh)
            nc.gpsimd.memset(a7[:, HID:HID + 1], 1.0)

            a4T = transp_aug(a4[:], HID, 'a4T')
            a7T = transp_aug(a7[:], HID, 'a7T')
            p1 = spool.tile([128, 256], F32, tag='sp')
            nc.tensor.matmul(p1[:, 0:A1], a4T[:], l6_sb[:],
                             start=True, stop=True)
            p2 = spool.tile([128, 256], F32, tag='sp')
            nc.tensor.matmul(p2[:, 0:A2], a7T[:], l9_sb[:],
                             start=True, stop=True)
            a5 = wpool.tile([P, A1], F32, tag='a5')
            nc.scalar.activation(a5[:], p1[:, 0:A1], ACTF.Tanh)
            sg = wpool.tile([P, A2], F32, tag='sg')
            nc.scalar.activation(sg[:], p2[:, 0:A2], ACTF.Tanh, scale=0.5)
            nc.vector.tensor_scalar(
                outb[:, it, A1:A1 + A2], sg[:], 0.5, 0.5,
                op0=ALU.mult, op1=ALU.add)
            nc.vector.select(outb[:, it, 0:A1], mask_sb[:, it, :],
                             a5[:], neg_sb[:])

        if rep_ctx is not None:
            rep_ctx.__exit__(None, None, None)
        outb2 = opool.tile([P, nt, A1 + A2], F32, tag='outb2')
        nc.scalar.copy(outb2[:], outb[:])
        nc.sync.dma_start(
            out=out1_d[:].rearrange('(t p) c -> p t c', p=P),
            in_=outb2[:, :, 0:A1])
        nc.sync.dma_start(
            out=out2_d[:].rearrange('(t p) c -> p t c', p=P),
            in_=outb2[:, :, A1:A1 + A2])

    nc.compile()
    return nc


def kernel(**inputs):
    in_maps = _host_prep(inputs)
    nc = build_program(BC)
    res = run_bass_kernel_spmd(nc, in_maps, list(range(NCORES))).results
    out1 = np.concatenate([r['out1'] for r in res], axis=0)
    out2 = np.concatenate([r['out2'] for r in res], axis=0)
    return out1, out2
